# revision 1
# baseline (speedup 1.0000x reference)
"""Trainium2 Bass kernel for nn_NodeModel (GNN message passing + node MLP), V2.

  agg = scatter_mean(edge_attr, col, N)            # [N, H]
  h   = concat([x, agg]) @ W1 + b1                 # [N, 2H]
  h   = LayerNorm(h) * gamma (+ beta=0)
  h   = PReLU(h)  (single shared a)
  out = h @ W2 + b2                                # [N, H]

V2 strategy (8 cores SPMD, no collectives):
  - Nodes degree-sorted desc, dealt round-robin by rank to the 8 cores, so
    every core sees the SAME per-position degree profile D[i] (padding
    ~0.03%).  Per-core node order = degree desc.
  - Edge attrs are host-scaled by 1/max(cnt,1) (folds the mean) and shipped
    feature-major [128, E_pad] f16, slot-major within each degree class:
    column = base + slot*n + node.  The segment sum for a class of degree d
    over n nodes is then a pairwise tree of (d-1) tensor_tensor adds on
    [128, n] stride-1 f16 slices - 2x DVE mode, zero PE work.  Degrees > 8
    are split into virtual nodes (<=8 each) plus a small 2nd-level tree.
    Trees are split DVE / GPSIMD to balance engines.
  - MLP feature-major in f16: W1 (mean-centered) 4 matmuls, variance via
    ones-matmul over h^2 (DVE square), rstd = exp(-0.5 ln(var+eps)),
    broadcast by rank-1 matmul; PReLU folded into two W2 matmuls with
    rhs (h, |h|) and rstd applied at the end (commutes past W2 and |.|).
  - f16 end-to-end: in/out DMA bytes halved vs f32.
"""
import os
import sys

sys.path.insert(0, "/opt/trn_rl_repo")
_HERE = os.path.dirname(os.path.abspath(__file__))
if _HERE not in sys.path:
    sys.path.insert(0, _HERE)

import numpy as np

import concourse.bass as bass
import concourse.tile as tile
from concourse import mybir
from concourse.mybir import AluOpType as alu
from concourse.mybir import ActivationFunctionType as act

F32 = mybir.dt.float32
F16 = mybir.dt.float16

N_CORES = 8
H = 128
NPC = 12800                    # nodes per core
N_PAD = N_CORES * NPC
MTILE = 512
NMT = NPC // MTILE             # 25
KMAX = 8                       # max tree window; larger degrees split
CHUNKMAX = 8192                # edge-columns per DMA chunk (16KB/partition)
GP_FRAC = 0.30                 # fraction of tree nodes handed to GPSIMD
EATTR_FP8 = False              # fp8 edge attrs (unsupported walrus path here)
F8 = mybir.dt.float8e4

# ---------------------------------------------------------------------------
# walrus workaround (same as baseline): single sync-wait per instruction +
# skip the crashy birverifier pass.
import bass_rust


def _split_multi_waits(nc):
    ctr = 0
    for f in nc.m.functions:
        for blk in f.blocks:
            insts = list(blk.instructions)
            new = []
            changed = False
            for inst in insts:
                si = inst.sync_info
                if si is not None and len(si.on_wait) > 1:
                    waits = list(si.on_wait)
                    for w in waits[:-1]:
                        ctr += 1
                        new.append(mybir.InstEventSemaphore(
                            name=f"wsplit_{ctr}", engine=inst.engine,
                            ins=[], outs=[],
                            sync_info=bass_rust.SyncInfo(on_wait=[w],
                                                         on_update=[]),
                        ))
                    si.on_wait = [waits[-1]]
                    changed = True
                new.append(inst)
            if changed:
                blk.instructions = new


def _fuse_single_waits(nc):
    for f in nc.m.functions:
        for blk in f.blocks:
            insts = list(blk.instructions)
            drop = set()
            pending = {}
            for i, inst in enumerate(insts):
                eng = inst.engine
                si = inst.sync_info
                tname = type(inst).__name__
                if (tname == "InstEventSemaphore" and si is not None
                        and len(si.on_wait) == 1 and len(si.on_update) == 0
                        and eng not in pending):
                    pending[eng] = (i, si.on_wait[0])
                    continue
                if eng in pending:
                    if si is not None and len(si.on_wait) > 0:
                        pending.pop(eng)
                    elif tname in ("InstEventSemaphore", "InstDrain",
                                   "InstNoOp", "InstCall", "InstBranch"):
                        pending.pop(eng)
                    else:
                        j, w = pending.pop(eng)
                        if si is None:
                            inst.sync_info = bass_rust.SyncInfo(
                                on_wait=[w], on_update=[])
                        else:
                            si.on_wait = [w]
                        drop.add(j)
            if drop:
                blk.instructions = [x for i, x in enumerate(insts)
                                    if i not in drop]


def _skip_birverifier():
    from concourse import bass_utils as bu
    from pathlib import Path

    if getattr(bu, "_nodemodel_noverify", False):
        return

    def bir_verify_and_optimise(tmpdir, inp="bir.json", outp="file.neff",
                                arch=None, *, dve_root=None):
        cmd = [
            bu.get_walrus_driver(),
            "--pass",
            "runtime_memory_reservation,lower_act,lower_dve,"
            "lower_ap_offset,codegen,neff_packager",
            "-i", inp,
            "--neff-output-filename", outp,
            "--enable-birsim=true",
            "--mem-mode=physical",
            "--policy=0",
            "--enable-ldw-opt=false",
            "--assign-static-dmas-to-sp=false",
            f"--dram-page-size={bu.aot_getenv('NEURON_SCRATCHPAD_PAGE_SIZE', '256')}",
            "--enable-neff-debug-info=true",
            "--jobs", "8",
            *bu.get_walrus_args(
                bu.get_bir_arch(tmpdir, inp) if arch is None else arch,
                tmpdir, dve_root=dve_root),
        ]
        result = bu.run_command(cmd, cwd=tmpdir)
        if result is not None:
            (Path(tmpdir) / "log.txt").write_text(result.stdout)
        return f"{tmpdir}/{outp}"

    bu.bir_verify_and_optimise = bir_verify_and_optimise
    bu._nodemodel_noverify = True


# ---------------------------------------------------------------------------
# f16 weight/constant buffer layout
_O16 = {}
_O32 = {}


def _layouts():
    off = 0
    def t16(name, n):
        nonlocal off
        _O16[name] = off
        off += n
    t16("w1a0", 128); t16("w1a1", 128)
    t16("w1b0", 128); t16("w1b1", 128)
    t16("w2v0", 128); t16("w2v1", 128)
    t16("w2u0", 128); t16("w2u1", 128)
    t16("er", 16)          # er_r [128,4], col r ones (var-MM lhsT)
    t16("sel", 512)        # sel_r [4,128], row r ones (bcast lhsT)
    n16 = off
    off = 0
    def t32(name, n):
        nonlocal off
        _O32[name] = off
        off += n
    t32("b1c0", 1); t32("b1c1", 1); t32("b2c", 1); t32("epsc", 1)
    return n16, off


W16C, W32C = _layouts()


def _build_wbufs(W1, b1, gamma, beta, prelu_a, W2, b2):
    a = float(np.asarray(prelu_a).reshape(-1)[0])
    W1 = np.asarray(W1, np.float32)
    W2 = np.asarray(W2, np.float32)
    b1 = np.asarray(b1, np.float32)
    b2 = np.asarray(b2, np.float32)
    gamma = np.asarray(gamma, np.float32)
    W1c = W1 - W1.mean(axis=1, keepdims=True)
    b1c = b1 - b1.mean()
    w2v = W2 * ((1.0 + a) / 2.0 * gamma)[:, None]
    w2u = W2 * ((1.0 - a) / 2.0 * np.abs(gamma))[:, None]

    w16 = np.zeros((128, W16C), np.float16)
    def s16(name, arr):
        w16[:, _O16[name]:_O16[name] + arr.shape[1]] = arr.astype(np.float16)
    s16("w1a0", W1c[0:128, 0:128]); s16("w1a1", W1c[0:128, 128:256])
    s16("w1b0", W1c[128:256, 0:128]); s16("w1b1", W1c[128:256, 128:256])
    s16("w2v0", w2v[0:128, :]); s16("w2v1", w2v[128:256, :])
    s16("w2u0", w2u[0:128, :]); s16("w2u1", w2u[128:256, :])
    for r_ in range(4):
        w16[:, _O16["er"] + 4 * r_ + r_] = 1.0          # er_r col r ones
        w16[r_, _O16["sel"] + 128 * r_:
            _O16["sel"] + 128 * (r_ + 1)] = 1.0         # sel_r row r ones

    w32 = np.zeros((128, W32C), np.float32)
    w32[:, _O32["b1c0"]] = b1c[0:128]
    w32[:, _O32["b1c1"]] = b1c[128:256]
    w32[:, _O32["b2c"]] = b2
    w32[:, _O32["epsc"]] = 1e-5
    return w16, w32


# ---------------------------------------------------------------------------
# Structure plan from the shared degree profile D[0..NPC-1] (non-increasing).
def _make_plan(D):
    D = np.asarray(D, np.int64)
    assert D.shape == (NPC,)
    # class runs (d, p0, p1)
    classes = []
    p = 0
    while p < NPC:
        d = int(D[p])
        q = p
        while q < NPC and D[q] == d:
            q += 1
        classes.append((d, p, q))
        p = q
    zlo = zhi = 0
    for d, p0, p1 in classes:
        if d == 0:
            zlo, zhi = p0, p1
            break

    # level-1 runs: (d, n, sp, kind, dst, cls_id, jj)
    #   kind 0 -> dst = agg position sp ; kind 1 -> dst = vscr column
    runs = []
    lvl2 = []          # (nv, n, p0, vq0)
    vq = 0
    for ci, (d, p0, p1) in enumerate(classes):
        n = p1 - p0
        if d == 0:
            continue
        if d <= KMAX:
            runs.append([d, n, p0, 0, p0, ci, 0])
        else:
            nv = -(-d // KMAX)
            base = d // nv
            rem = d % nv
            # slot j has degree base+1 for j<rem else base
            for j in range(nv):
                dv = base + 1 if j < rem else base
                runs.append([dv, n, p0, 1, vq + j * n, ci, j])
            lvl2.append((nv, n, p0, vq))
            vq += nv * n
    NV = vq

    # split runs into sub-runs (node ranges) so d*n <= CHUNKMAX, assign src.
    # Each node range is further split DVE / GPSIMD (GP_FRAC of nodes to
    # GPSIMD) so both engines chew every chunk concurrently.
    subruns = []       # (d, n, sp, src, dst, kind, cls, jj, eng)
    src = 0
    for d, n, sp, kind, dst, ci, jj in runs:
        nmax = max(1, CHUNKMAX // d)
        q = 0
        while q < n:
            nn = min(nmax, n - q)
            ng = int(round(nn * GP_FRAC))
            nd = nn - ng
            if nd > 0:
                subruns.append([d, nd, sp + q, src, dst + q, kind, ci, jj, 0])
                src += d * nd
            if ng > 0:
                subruns.append([d, ng, sp + q + nd, src, dst + q + nd,
                                kind, ci, jj, 1])
                src += d * ng
            q += nn
    E_pad = src

    # chunks of consecutive sub-runs
    chunks = []        # (src0, ncols, (sub idx...))
    cur = []
    cur0 = 0
    cols = 0
    for i, sr in enumerate(subruns):
        c = sr[0] * sr[1]
        if cur and cols + c > CHUNKMAX:
            chunks.append((cur0, cols, tuple(cur)))
            cur = []
            cols = 0
        if not cur:
            cur0 = sr[3]
        cur.append(i)
        cols += c
    if cur:
        chunks.append((cur0, cols, tuple(cur)))

    # schedule: chunks in order; lvl2 right after the last chunk feeding it;
    # MLP tiles as soon as their agg range is fully written.
    last_chunk_of_class = {}
    for ci_ch, (_, _, idxs) in enumerate(chunks):
        for i in idxs:
            last_chunk_of_class[subruns[i][6]] = ci_ch
    lvl2_after = {}
    for li, (nv, n, p0, vq0) in enumerate(lvl2):
        # class id of this lvl2 = the class whose p0 matches
        for ci, (d, cp0, cp1) in enumerate(classes):
            if cp0 == p0 and d > KMAX:
                lvl2_after.setdefault(last_chunk_of_class[ci], []).append(li)
                break

    # coverage: position p ready after event; direct positions after their
    # chunk, virtual positions after their lvl2
    ready_at = np.full(NPC, -1, np.int64)    # chunk index after which ready
    for ci_ch, (_, _, idxs) in enumerate(chunks):
        for i in idxs:
            d, nn, sp, _, _, kind, ci, jj, _ = subruns[i]
            if kind == 0:
                ready_at[sp:sp + nn] = np.maximum(ready_at[sp:sp + nn], ci_ch)
    for li, (nv, n, p0, vq0) in enumerate(lvl2):
        # ready after the chunk that its lvl2 is scheduled after
        for ci_ch, lis in lvl2_after.items():
            if li in lis:
                ready_at[p0:p0 + n] = ci_ch
    ready_at[zlo:zhi] = -1                   # ready from start

    # MLP emitted in groups of 4 tiles (stats batched per group)
    ngroups = -(-NMT // 4)
    sched = []
    g = 0
    for ci_ch in range(len(chunks)):
        sched.append(("c", ci_ch))
        for li in lvl2_after.get(ci_ch, []):
            sched.append(("v", li))
        while g < ngroups and np.all(
                ready_at[:min(4 * (g + 1) * MTILE, NPC)] <= ci_ch):
            sched.append(("g", g))
            g += 1
    while g < ngroups:
        sched.append(("g", g))
        g += 1

    return dict(
        E_pad=E_pad, NV=NV, zrange=(zlo, zhi),
        classes=tuple(map(tuple, classes)),
        subruns=tuple(map(tuple, subruns)),
        chunks=tuple(chunks),
        lvl2=tuple(lvl2),
        sched=tuple(sched),
    )


def _plan_key(plan):
    return (plan["E_pad"], plan["NV"], plan["zrange"], plan["subruns"],
            plan["chunks"], plan["lvl2"], plan["sched"],
            plan.get("b2z", False))


# ---------------------------------------------------------------------------
def _build_program(plan, reps=1, unroll=1):
    import contextlib
    _skip_birverifier()
    E_pad, NV = plan["E_pad"], plan["NV"]
    zlo, zhi = plan["zrange"]
    subruns, chunks, lvl2 = plan["subruns"], plan["chunks"], plan["lvl2"]
    sched = plan["sched"]

    nc = bass.Bass("TRN2", target_bir_lowering=False, debug=False,
                   num_devices=N_CORES)
    d_eattr = nc.dram_tensor("eattr", [128, E_pad],
                             F8 if EATTR_FP8 else F16,
                             kind="ExternalInput").ap()
    d_xT = nc.dram_tensor("xT", [128, NPC], F16, kind="ExternalInput").ap()
    d_w16 = nc.dram_tensor("w16", [128, W16C], F16,
                           kind="ExternalInput").ap()
    d_w32 = nc.dram_tensor("w32", [128, W32C], F32,
                           kind="ExternalInput").ap()
    d_outT = nc.dram_tensor("outT", [128, NPC], F16,
                            kind="ExternalOutput").ap()

    with tile.TileContext(nc) as tc:
        with tc.tile_pool(name="const", bufs=1) as constp, \
             tc.tile_pool(name="chp", bufs=3) as chp, \
             tc.tile_pool(name="trp", bufs=8) as trp, \
             tc.tile_pool(name="hp", bufs=10) as hp, \
             tc.tile_pool(name="sqp", bufs=4) as sqp, \
             tc.tile_pool(name="up", bufs=10) as up, \
             tc.tile_pool(name="omp", bufs=3) as omp, \
             tc.tile_pool(name="osbp", bufs=3) as osbp, \
             tc.tile_pool(name="statp", bufs=4) as statp, \
             tc.tile_pool(name="ps_h", bufs=3, space="PSUM") as ps_h, \
             tc.tile_pool(name="ps_st", bufs=3, space="PSUM") as ps_st, \
             tc.tile_pool(name="ps_po", bufs=2, space="PSUM") as ps_po:

            w16 = constp.tile([128, W16C], F16)
            nc.sync.dma_start(w16[:], d_w16)
            w32 = constp.tile([128, W32C], F32)
            nc.sync.dma_start(w32[:], d_w32)
            agg = constp.tile([128, NPC], F16)
            vscr = constp.tile([128, max(NV, 1)], F16)
            xfull = constp.tile([128, NPC], F16)

            if zhi > zlo:
                nc.gpsimd.memset(agg[:, zlo:zhi], 0.0)

            def W16(name, n=128):
                return w16[:, _O16[name]:_O16[name] + n]

            def W32(name):
                return w32[:, _O32[name]:_O32[name] + 1]


            uid = [0]

            def emit_tree(eng, blocks, dst, n):
                cur = list(blocks)
                if len(cur) == 1:
                    eng.tensor_copy(dst, cur[0])
                    return
                while len(cur) > 1:
                    nxt = []
                    i = 0
                    while i + 1 < len(cur):
                        if len(cur) == 2:
                            o = dst
                        else:
                            uid[0] += 1
                            t = trp.tile([128, n], F16,
                                         name=f"tr{uid[0]}", tag="tr")
                            o = t[:]
                        eng.tensor_tensor(o, cur[i], cur[i + 1], alu.add)
                        nxt.append(o)
                        i += 2
                    if i < len(cur):
                        nxt.append(cur[i])
                    cur = nxt

            def emit_chunk(ci_ch):
                src0, ncols, idxs = chunks[ci_ch]
                uid[0] += 1
                ch = chp.tile([128, ncols], F16, name=f"ch{uid[0]}", tag="ch")
                if EATTR_FP8:
                    # gpsimd DMA casts fp8 -> f16 in flight (HBM reads halved)
                    nc.gpsimd.dma_start(ch[:], d_eattr[:, src0:src0 + ncols])
                else:
                    nc.sync.dma_start(ch[:], d_eattr[:, src0:src0 + ncols])
                for i in idxs:
                    d, n, sp, src, dst, kind, cidx, jj, eng_id = subruns[i]
                    eng = nc.gpsimd if eng_id else nc.vector
                    off = src - src0
                    blocks = [ch[:, off + j * n: off + (j + 1) * n]
                              for j in range(d)]
                    if kind == 0:
                        out = agg[:, sp:sp + n]
                    else:
                        out = vscr[:, dst:dst + n]
                    emit_tree(eng, blocks, out, n)

            def emit_lvl2(li):
                nv, n, p0, vq0 = lvl2[li]
                blocks = [vscr[:, vq0 + j * n: vq0 + (j + 1) * n]
                          for j in range(nv)]
                emit_tree(nc.vector, blocks, agg[:, p0:p0 + n], n)

            U16 = mybir.dt.uint16

            def emit_mlp_group(g):
                m0 = 4 * g
                m1 = min(m0 + 4, NMT)
                G = m1 - m0
                pb = ps_st.tile([4, MTILE], F32, tag="st", name=f"pb_{g}")
                hs, us = [], []
                # phase 1: W1 + h/sq/u + variance rows for the whole group
                for r in range(G):
                    m = m0 + r
                    sl = slice(m * MTILE, (m + 1) * MTILE)
                    xt = xfull[:, sl]
                    aggm = agg[:, sl]
                    ph0 = ps_h.tile([128, MTILE], F32, tag="ph",
                                    name=f"ph0_{m}")
                    ph1 = ps_h.tile([128, MTILE], F32, tag="ph",
                                    name=f"ph1_{m}")
                    nc.tensor.matmul(ph0[:], W16("w1a0"), xt, start=True,
                                     stop=False)
                    nc.tensor.matmul(ph0[:], W16("w1b0"), aggm, start=False,
                                     stop=True)
                    nc.tensor.matmul(ph1[:], W16("w1a1"), xt, start=True,
                                     stop=False)
                    nc.tensor.matmul(ph1[:], W16("w1b1"), aggm, start=False,
                                     stop=True)
                    h0 = hp.tile([128, MTILE], F16, tag="h", name=f"h0_{m}")
                    h1 = hp.tile([128, MTILE], F16, tag="h", name=f"h1_{m}")
                    nc.scalar.activation(h0[:], ph0[:], act.Identity,
                                         bias=W32("b1c0"))
                    nc.scalar.activation(h1[:], ph1[:], act.Identity,
                                         bias=W32("b1c1"))
                    sq0 = sqp.tile([128, MTILE], F16, tag="sq",
                                   name=f"sq0_{m}")
                    sq1 = sqp.tile([128, MTILE], F16, tag="sq",
                                   name=f"sq1_{m}")
                    nc.vector.tensor_tensor(sq0[:], h0[:], h0[:], alu.mult)
                    nc.vector.tensor_tensor(sq1[:], h1[:], h1[:], alu.mult)
                    u0 = up.tile([128, MTILE], F16, tag="u", name=f"u0_{m}")
                    u1 = up.tile([128, MTILE], F16, tag="u", name=f"u1_{m}")
                    # |h| on f16 = clear sign bit (no ABS in mybir's alu set;
                    # bitvec ops need integer dtypes -> bitcast to u16)
                    nc.vector.tensor_scalar(u0[:].bitcast(U16),
                                            h0[:].bitcast(U16), 0x7FFF, None,
                                            alu.bitwise_and)
                    nc.vector.tensor_scalar(u1[:].bitcast(U16),
                                            h1[:].bitcast(U16), 0x7FFF, None,
                                            alu.bitwise_and)
                    er = w16[:, _O16["er"] + 4 * r:_O16["er"] + 4 * r + 4]
                    nc.tensor.matmul(pb[0:4, :], er, sq0[:],
                                     start=(r == 0), stop=False)
                    nc.tensor.matmul(pb[0:4, :], er, sq1[:],
                                     start=False, stop=(r == G - 1))
                    hs.append((h0, h1))
                    us.append((u0, u1))
                # batched stats: rstd rows for the group
                yrow = statp.tile([4, MTILE], F32, tag="yr", name=f"yr_{g}")
                nc.scalar.activation(yrow[0:G, :], pb[0:G, :], act.Ln,
                                     scale=1.0 / 256.0,
                                     bias=w32[0:G, _O32["epsc"]:
                                              _O32["epsc"] + 1])
                rrow = statp.tile([4, MTILE], F16, tag="rr", name=f"rr_{g}")
                nc.scalar.activation(rrow[0:G, :], yrow[0:G, :], act.Exp,
                                     scale=-0.5)
                # phase 2: broadcast rstd, W2, scale, bias, store
                for r in range(G):
                    m = m0 + r
                    sl = slice(m * MTILE, (m + 1) * MTILE)
                    h0, h1 = hs[r]
                    u0, u1 = us[r]
                    sel = w16[0:4, _O16["sel"] + 128 * r:
                              _O16["sel"] + 128 * (r + 1)]
                    pr = ps_st.tile([128, MTILE], F32, tag="st",
                                    name=f"pr_{m}")
                    nc.tensor.matmul(pr[:], sel, rrow[0:4, :], start=True,
                                     stop=True)
                    rr_sb = omp.tile([128, MTILE], F16, tag="rrs",
                                     name=f"rrs_{m}")
                    nc.scalar.activation(rr_sb[:], pr[:], act.Copy)
                    po = ps_po.tile([128, MTILE], F32, tag="po",
                                    name=f"po_{m}")
                    nc.tensor.matmul(po[:], W16("w2v0"), h0[:], start=True,
                                     stop=False)
                    nc.tensor.matmul(po[:], W16("w2v1"), h1[:], start=False,
                                     stop=False)
                    nc.tensor.matmul(po[:], W16("w2u0"), u0[:], start=False,
                                     stop=False)
                    nc.tensor.matmul(po[:], W16("w2u1"), u1[:], start=False,
                                     stop=True)
                    osb = osbp.tile([128, MTILE], F16, tag="osb",
                                    name=f"osb_{m}")
                    if plan.get("b2z"):
                        # b2 == 0: the rstd multiply writes the output tile
                        nc.vector.tensor_tensor(osb[:], po[:], rr_sb[:],
                                                alu.mult)
                    else:
                        om = omp.tile([128, MTILE], F16, tag="om",
                                      name=f"om_{m}")
                        nc.vector.tensor_tensor(om[:], po[:], rr_sb[:],
                                                alu.mult)
                        nc.vector.tensor_scalar(osb[:], om[:], W32("b2c"),
                                                None, alu.add)
                    nc.sync.dma_start(d_outT[:, sl], osb[:])

            def emit_rep():
                for xi in range(4):
                    sl = slice(xi * (NPC // 4), (xi + 1) * (NPC // 4))
                    nc.sync.dma_start(xfull[:, sl], d_xT[:, sl])
                for kind_it, idx in sched:
                    if kind_it == "c":
                        emit_chunk(idx)
                    elif kind_it == "v":
                        emit_lvl2(idx)
                    else:
                        emit_mlp_group(idx)

            with nc.allow_low_precision("f16 pipeline"):
                if unroll > 1:
                    for _ in range(unroll):
                        emit_rep()
                else:
                    rep_ctx = (tc.For_i(0, reps, 1) if reps > 1
                               else contextlib.nullcontext())
                    rep_ctx.__enter__()
                    emit_rep()
                    rep_ctx.__exit__(None, None, None)

    _split_multi_waits(nc)
    _fuse_single_waits(nc)
    return nc


# ---------------------------------------------------------------------------
class _Runner:
    """Persistent executor: jit once, keep inputs on device."""

    def __init__(self, nc):
        import jax
        from jax.experimental.shard_map import shard_map
        from jax.sharding import Mesh, PartitionSpec, NamedSharding
        from concourse import bass2jax
        from concourse import mybir as _mb

        bass2jax.install_neuronx_cc_hook()
        self.nc = nc
        in_names, out_names, out_avals = [], [], []
        partition_name = (nc.partition_id_tensor.name
                          if nc.partition_id_tensor else None)
        for alloc in nc.m.functions[0].allocations:
            if not isinstance(alloc, _mb.MemoryLocationSet):
                continue
            name = alloc.memorylocations[0].name
            if alloc.kind == "ExternalInput":
                if name != partition_name:
                    in_names.append(name)
            elif alloc.kind == "ExternalOutput":
                out_names.append(name)
                out_avals.append(jax.core.ShapedArray(
                    tuple(alloc.tensor_shape), _mb.dt.np(alloc.dtype)))
        self.in_names, self.out_names, self.out_avals = \
            in_names, out_names, out_avals
        n_params, n_outs = len(in_names), len(out_avals)
        all_in = list(in_names) + list(out_names)
        if partition_name is not None:
            all_in.append(partition_name)

        def _body(*args):
            operands = list(args)
            if partition_name is not None:
                operands.append(bass2jax.partition_id_tensor())
            return tuple(bass2jax._bass_exec_p.bind(
                *operands,
                out_avals=tuple(out_avals),
                in_names=tuple(all_in),
                out_names=tuple(out_names),
                lowering_input_output_aliases=(),
                sim_require_finite=True,
                sim_require_nnan=True,
                nc=nc,
            ))

        devices = jax.devices()[:N_CORES]
        mesh = Mesh(np.asarray(devices), ("core",))
        self.mesh = mesh
        self.sharding = NamedSharding(mesh, PartitionSpec("core"))
        in_specs = (PartitionSpec("core"),) * (n_params + n_outs)
        out_specs = (PartitionSpec("core"),) * n_outs
        donate = tuple(range(n_params, n_params + n_outs))
        self.fn = jax.jit(
            shard_map(_body, mesh=mesh, in_specs=in_specs,
                      out_specs=out_specs, check_rep=False),
            donate_argnums=donate, keep_unused=True)
        self._zero = jax.jit(
            lambda: tuple(
                jax.numpy.zeros((N_CORES * a.shape[0], *a.shape[1:]), a.dtype)
                for a in out_avals),
            out_shardings=tuple(self.sharding for _ in out_avals))
        self._dev_inputs = None
        self._dev_key = None

    def put_inputs(self, in_maps):
        import jax
        key = tuple(id(m[n]) for m in in_maps for n in self.in_names)
        if self._dev_key == key and self._dev_inputs is not None:
            return
        concat = [np.concatenate([np.asarray(m[n]) for m in in_maps], axis=0)
                  for n in self.in_names]
        self._dev_inputs = [jax.device_put(a, self.sharding) for a in concat]
        for a in self._dev_inputs:
            a.block_until_ready()
        self._dev_key = key

    def execute(self):
        zeros = self._zero()
        outs = self.fn(*self._dev_inputs, *zeros)
        return outs

    def run(self, in_maps):
        self.put_inputs(in_maps)
        outs = self.execute()
        res = []
        for c in range(N_CORES):
            res.append({
                name: np.asarray(outs[i]).reshape(
                    N_CORES, *self.out_avals[i].shape)[c]
                for i, name in enumerate(self.out_names)})
        return res

    def time_once(self):
        import time as _t
        zeros = self._zero()
        for z in zeros:
            z.block_until_ready()
        t0 = _t.perf_counter()
        outs = self.fn(*self._dev_inputs, *zeros)
        for o in outs:
            o.block_until_ready()
        return _t.perf_counter() - t0


_CACHE = {}


def _prepare(x, edge_index, edge_attr, W1, b1, gamma, beta, prelu_a, W2, b2):
    N, E = x.shape[0], edge_attr.shape[0]
    assert np.all(np.asarray(beta) == 0.0), "kernel specialized for beta=0"
    x = np.asarray(x, np.float32)
    edge_attr = np.asarray(edge_attr, np.float32)
    col = np.asarray(edge_index)[1].astype(np.int64)

    cnt = np.bincount(col, minlength=N_PAD).astype(np.int64)
    inv = (1.0 / np.maximum(cnt, 1.0)).astype(np.float32)

    # Custom class order tuned for pipeline ramp: zero-degree first (agg is
    # memset -> MLP group 0 starts immediately), mid degrees next, the
    # two-level virtual classes (d>KMAX, longest latency chains) in the
    # middle, and the shallowest trees (d=3,2,1) last so the final MLP
    # group's agg is ready right after the last (small) chunk.
    dmax = int(cnt.max())
    rank = np.empty(dmax + 1, np.int64)
    for d in range(dmax + 1):
        if d == 0:
            rank[d] = 0
        elif 4 <= d <= KMAX:
            rank[d] = 1 + (KMAX - d)           # 8,7,6,5,4
        elif d > KMAX:
            rank[d] = 100 + d                  # virtuals, middle
        else:
            rank[d] = 1000 + (3 - d)           # 3,2,1 at the tail
    order = np.argsort(rank[cnt], kind="stable")
    r = np.arange(N_PAD)
    new_of_old = np.empty(N_PAD, np.int64)
    new_of_old[order] = (r % N_CORES) * NPC + r // N_CORES
    D = cnt[order].reshape(NPC, N_CORES).max(axis=1)   # shared profile

    plan = _make_plan(D)
    # NB: skipping the (zero) b2-add measurably HURTS the schedule - the ts
    # decouples the psum-reading multiply from the out-DMA. Keep it.
    plan["b2z"] = False
    E_pad = plan["E_pad"]

    # per-edge (core, pos, slot)
    new = new_of_old[col]
    core_e = new // NPC
    pos_e = new % NPC
    order_e = np.argsort(new, kind="stable")
    sn = new[order_e]
    change = np.r_[True, sn[1:] != sn[:-1]]
    startidx = np.maximum.accumulate(np.where(change, np.arange(E), 0))
    slot = np.empty(E, np.int64)
    slot[order_e] = np.arange(E) - startidx

    # per-edge column via sub-run lookup
    classes = plan["classes"]
    colx = np.full(E, -1, np.int64)
    # precompute virtual split (j, e_in) for edges in split classes
    jj_e = np.zeros(E, np.int64)
    ein_e = slot.copy()
    for ci, (d, p0, p1) in enumerate(classes):
        if d <= KMAX:
            continue
        nv = -(-d // KMAX)
        base = d // nv
        rem = d % nv
        cmask = (pos_e >= p0) & (pos_e < p1)
        e = slot[cmask]
        b1_ = rem * (base + 1)
        jv = np.where(e < b1_, e // (base + 1), rem + (e - b1_) // base)
        ev = np.where(e < b1_, e % (base + 1), (e - b1_) % base)
        jj_e[cmask] = jv
        ein_e[cmask] = ev
    for d, n, sp, src, dst, kind, ci, jj, eng in plan["subruns"]:
        m = (pos_e >= sp) & (pos_e < sp + n) & (jj_e == jj)
        colx[m] = src + ein_e[m] * n + (pos_e[m] - sp)
    assert (colx >= 0).all()

    edtype = mybir.dt.np(F8) if EATTR_FP8 else np.float16
    val = (edge_attr * inv[col][:, None]).astype(edtype)
    buf = np.zeros((N_CORES, E_pad, H), edtype)
    buf[core_e, colx] = val
    eattrT = np.ascontiguousarray(buf.transpose(0, 2, 1))

    xp = np.zeros((N_PAD, H), np.float32)
    xp[new_of_old[:N]] = x
    xT = np.ascontiguousarray(
        xp.reshape(N_CORES, NPC, H).transpose(0, 2, 1)).astype(np.float16)

    w16, w32 = _build_wbufs(W1, b1, gamma, beta, prelu_a, W2, b2)

    in_maps = [
        {"eattr": eattrT[c], "xT": xT[c], "w16": w16, "w32": w32}
        for c in range(N_CORES)
    ]
    return plan, in_maps, new_of_old


def get_runner(plan, reps=1):
    ck = (_plan_key(plan), reps)
    runner = _CACHE.get(ck)
    if runner is None:
        nc = _build_program(plan, reps=reps)
        runner = _Runner(nc)
        _CACHE[ck] = runner
    return runner


def kernel(x, edge_index, edge_attr, W1, b1, gamma, beta, prelu_a, W2, b2,
           **_unused):
    N = x.shape[0]
    plan, in_maps, new_of_old = _prepare(x, edge_index, edge_attr, W1, b1,
                                         gamma, beta, prelu_a, W2, b2)
    runner = get_runner(plan)
    res = runner.run(in_maps)
    outT = np.stack([r["outT"] for r in res])           # [8,128,NPC] f16
    out = outT.transpose(0, 2, 1).reshape(N_PAD, H).astype(np.float32)
    out = out[new_of_old[:N]]
    return np.ascontiguousarray(out)


if __name__ == "__main__":
    rng = np.random.default_rng(0)
    N, E = N_PAD, 60000
    x = rng.standard_normal((N, H), dtype=np.float32)
    ei = rng.integers(0, N, size=(2, E)).astype(np.int64)
    ea = rng.standard_normal((E, H), dtype=np.float32)
    W1 = rng.standard_normal((2 * H, 2 * H), dtype=np.float32) / 16
    b1 = np.zeros(2 * H, np.float32)
    g = np.ones(2 * H, np.float32)
    be = np.zeros(2 * H, np.float32)
    a = np.full(1, 0.25, np.float32)
    W2 = rng.standard_normal((2 * H, H), dtype=np.float32) / 16
    b2 = np.zeros(H, np.float32)
    out = kernel(x, ei, ea, W1, b1, g, be, a, W2, b2)
    print("out", out.shape, out.dtype, np.abs(out).mean())



# revision 75
# speedup vs baseline: 3.2501x; 3.2501x over previous
"""Trainium2 Bass kernel for nn_NodeModel (GNN message passing + node MLP), V2.

  agg = scatter_mean(edge_attr, col, N)            # [N, H]
  h   = concat([x, agg]) @ W1 + b1                 # [N, 2H]
  h   = LayerNorm(h) * gamma (+ beta=0)
  h   = PReLU(h)  (single shared a)
  out = h @ W2 + b2                                # [N, H]

V2 strategy (8 cores SPMD, no collectives):
  - Nodes degree-sorted desc, dealt round-robin by rank to the 8 cores, so
    every core sees the SAME per-position degree profile D[i] (padding
    ~0.03%).  Per-core node order = degree desc.
  - Edge attrs are host-scaled by 1/max(cnt,1) (folds the mean) and shipped
    feature-major [128, E_pad] f16, slot-major within each degree class:
    column = base + slot*n + node.  The segment sum for a class of degree d
    over n nodes is then a pairwise tree of (d-1) tensor_tensor adds on
    [128, n] stride-1 f16 slices - 2x DVE mode, zero PE work.  Degrees > 8
    are split into virtual nodes (<=8 each) plus a small 2nd-level tree.
    Trees are split DVE / GPSIMD to balance engines.
  - MLP feature-major in f16: W1 (mean-centered) 4 matmuls, variance via
    ones-matmul over h^2 (DVE square), rstd = exp(-0.5 ln(var+eps)),
    broadcast by rank-1 matmul; PReLU folded into two W2 matmuls with
    rhs (h, |h|) and rstd applied at the end (commutes past W2 and |.|).
  - f16 end-to-end: in/out DMA bytes halved vs f32.
"""
import os
import sys

sys.path.insert(0, "/opt/trn_rl_repo")
_HERE = os.path.dirname(os.path.abspath(__file__))
if _HERE not in sys.path:
    sys.path.insert(0, _HERE)

import numpy as np

import concourse.bass as bass
import concourse.tile as tile
from concourse import mybir
from concourse.mybir import AluOpType as alu
from concourse.mybir import ActivationFunctionType as act

F32 = mybir.dt.float32
F16 = mybir.dt.float16

N_CORES = 8
H = 128
NPC = 12800                    # nodes per core
N_PAD = N_CORES * NPC
MTILE = 512
NMT = NPC // MTILE             # 25
KMAX = 8                       # max tree window; larger degrees split
CHUNKMAX = 8192                # edge-columns per DMA chunk (16KB/partition)
GP_FRAC = 0.30                 # fraction of tree nodes handed to GPSIMD
EATTR_FP8 = False              # fp8 edge attrs (unsupported walrus path here)
F8 = mybir.dt.float8e4

# ---------------------------------------------------------------------------
# walrus workaround (same as baseline): single sync-wait per instruction +
# skip the crashy birverifier pass.
import bass_rust


def _split_multi_waits(nc):
    ctr = 0
    for f in nc.m.functions:
        for blk in f.blocks:
            insts = list(blk.instructions)
            new = []
            changed = False
            for inst in insts:
                si = inst.sync_info
                if si is not None and len(si.on_wait) > 1:
                    waits = list(si.on_wait)
                    for w in waits[:-1]:
                        ctr += 1
                        new.append(mybir.InstEventSemaphore(
                            name=f"wsplit_{ctr}", engine=inst.engine,
                            ins=[], outs=[],
                            sync_info=bass_rust.SyncInfo(on_wait=[w],
                                                         on_update=[]),
                        ))
                    si.on_wait = [waits[-1]]
                    changed = True
                new.append(inst)
            if changed:
                blk.instructions = new


def _fuse_single_waits(nc):
    for f in nc.m.functions:
        for blk in f.blocks:
            insts = list(blk.instructions)
            drop = set()
            pending = {}
            for i, inst in enumerate(insts):
                eng = inst.engine
                si = inst.sync_info
                tname = type(inst).__name__
                if (tname == "InstEventSemaphore" and si is not None
                        and len(si.on_wait) == 1 and len(si.on_update) == 0
                        and eng not in pending):
                    pending[eng] = (i, si.on_wait[0])
                    continue
                if eng in pending:
                    if si is not None and len(si.on_wait) > 0:
                        pending.pop(eng)
                    elif tname in ("InstEventSemaphore", "InstDrain",
                                   "InstNoOp", "InstCall", "InstBranch"):
                        pending.pop(eng)
                    else:
                        j, w = pending.pop(eng)
                        if si is None:
                            inst.sync_info = bass_rust.SyncInfo(
                                on_wait=[w], on_update=[])
                        else:
                            si.on_wait = [w]
                        drop.add(j)
            if drop:
                blk.instructions = [x for i, x in enumerate(insts)
                                    if i not in drop]


def _skip_birverifier():
    from concourse import bass_utils as bu
    from pathlib import Path

    if getattr(bu, "_nodemodel_noverify", False):
        return

    def bir_verify_and_optimise(tmpdir, inp="bir.json", outp="file.neff",
                                arch=None, *, dve_root=None):
        cmd = [
            bu.get_walrus_driver(),
            "--pass",
            "runtime_memory_reservation,lower_act,lower_dve,"
            "lower_ap_offset,codegen,neff_packager",
            "-i", inp,
            "--neff-output-filename", outp,
            "--enable-birsim=true",
            "--mem-mode=physical",
            "--policy=0",
            "--enable-ldw-opt=false",
            "--assign-static-dmas-to-sp=false",
            f"--dram-page-size={bu.aot_getenv('NEURON_SCRATCHPAD_PAGE_SIZE', '256')}",
            "--enable-neff-debug-info=true",
            "--jobs", "8",
            *bu.get_walrus_args(
                bu.get_bir_arch(tmpdir, inp) if arch is None else arch,
                tmpdir, dve_root=dve_root),
        ]
        result = bu.run_command(cmd, cwd=tmpdir)
        if result is not None:
            (Path(tmpdir) / "log.txt").write_text(result.stdout)
        return f"{tmpdir}/{outp}"

    bu.bir_verify_and_optimise = bir_verify_and_optimise
    bu._nodemodel_noverify = True


# ---------------------------------------------------------------------------
# f16 weight/constant buffer layout
_O16 = {}
_O32 = {}


def _layouts():
    off = 0
    def t16(name, n):
        nonlocal off
        _O16[name] = off
        off += n
    t16("w1a0", 128); t16("w1a1", 128)
    t16("w1b0", 128); t16("w1b1", 128)
    t16("w2v0", 128); t16("w2v1", 128)
    t16("w2u0", 128); t16("w2u1", 128)
    t16("er", 16)          # er_r [128,4], col r ones (var-MM lhsT)
    t16("sel", 512)        # sel_r [4,128], row r ones (bcast lhsT)
    n16 = off
    off = 0
    def t32(name, n):
        nonlocal off
        _O32[name] = off
        off += n
    t32("b1c0", 1); t32("b1c1", 1); t32("b2c", 1); t32("epsc", 1)
    return n16, off


W16C, W32C = _layouts()


def _build_wbufs(W1, b1, gamma, beta, prelu_a, W2, b2):
    a = float(np.asarray(prelu_a).reshape(-1)[0])
    W1 = np.asarray(W1, np.float32)
    W2 = np.asarray(W2, np.float32)
    b1 = np.asarray(b1, np.float32)
    b2 = np.asarray(b2, np.float32)
    gamma = np.asarray(gamma, np.float32)
    W1c = W1 - W1.mean(axis=1, keepdims=True)
    b1c = b1 - b1.mean()
    w2v = W2 * ((1.0 + a) / 2.0 * gamma)[:, None]
    w2u = W2 * ((1.0 - a) / 2.0 * np.abs(gamma))[:, None]

    w16 = np.zeros((128, W16C), np.float16)
    def s16(name, arr):
        w16[:, _O16[name]:_O16[name] + arr.shape[1]] = arr.astype(np.float16)
    s16("w1a0", W1c[0:128, 0:128]); s16("w1a1", W1c[0:128, 128:256])
    s16("w1b0", W1c[128:256, 0:128]); s16("w1b1", W1c[128:256, 128:256])
    s16("w2v0", w2v[0:128, :]); s16("w2v1", w2v[128:256, :])
    s16("w2u0", w2u[0:128, :]); s16("w2u1", w2u[128:256, :])
    for r_ in range(4):
        w16[:, _O16["er"] + 4 * r_ + r_] = 1.0          # er_r col r ones
        w16[r_, _O16["sel"] + 128 * r_:
            _O16["sel"] + 128 * (r_ + 1)] = 1.0         # sel_r row r ones

    w32 = np.zeros((128, W32C), np.float32)
    w32[:, _O32["b1c0"]] = b1c[0:128]
    w32[:, _O32["b1c1"]] = b1c[128:256]
    w32[:, _O32["b2c"]] = b2
    w32[:, _O32["epsc"]] = 1e-5
    return w16, w32


# ---------------------------------------------------------------------------
# Structure plan from the shared degree profile D[0..NPC-1] (non-increasing).
def _make_plan(D):
    D = np.asarray(D, np.int64)
    assert D.shape == (NPC,)
    # class runs (d, p0, p1)
    classes = []
    p = 0
    while p < NPC:
        d = int(D[p])
        q = p
        while q < NPC and D[q] == d:
            q += 1
        classes.append((d, p, q))
        p = q
    zlo = zhi = 0
    for d, p0, p1 in classes:
        if d == 0:
            zlo, zhi = p0, p1
            break

    # level-1 runs: (d, n, sp, kind, dst, cls_id, jj)
    #   kind 0 -> dst = agg position sp ; kind 1 -> dst = vscr column
    runs = []
    lvl2 = []          # (nv, n, p0, vq0)
    vq = 0
    for ci, (d, p0, p1) in enumerate(classes):
        n = p1 - p0
        if d == 0:
            continue
        if d <= KMAX:
            runs.append([d, n, p0, 0, p0, ci, 0])
        else:
            nv = -(-d // KMAX)
            base = d // nv
            rem = d % nv
            # slot j has degree base+1 for j<rem else base
            for j in range(nv):
                dv = base + 1 if j < rem else base
                runs.append([dv, n, p0, 1, vq + j * n, ci, j])
            lvl2.append((nv, n, p0, vq))
            vq += nv * n
    NV = vq

    # split runs into sub-runs (node ranges) so d*n <= CHUNKMAX, assign src.
    # Each node range is further split DVE / GPSIMD (GP_FRAC of nodes to
    # GPSIMD) so both engines chew every chunk concurrently.
    subruns = []       # (d, n, sp, src, dst, kind, cls, jj, eng)
    src = 0
    for d, n, sp, kind, dst, ci, jj in runs:
        nmax = max(1, CHUNKMAX // d)
        q = 0
        while q < n:
            nn = min(nmax, n - q)
            ng = int(round(nn * GP_FRAC))
            nd = nn - ng
            if nd > 0:
                subruns.append([d, nd, sp + q, src, dst + q, kind, ci, jj, 0])
                src += d * nd
            if ng > 0:
                subruns.append([d, ng, sp + q + nd, src, dst + q + nd,
                                kind, ci, jj, 1])
                src += d * ng
            q += nn
    E_pad = src

    # chunks of consecutive sub-runs
    chunks = []        # (src0, ncols, (sub idx...))
    cur = []
    cur0 = 0
    cols = 0
    for i, sr in enumerate(subruns):
        c = sr[0] * sr[1]
        if cur and cols + c > CHUNKMAX:
            chunks.append((cur0, cols, tuple(cur)))
            cur = []
            cols = 0
        if not cur:
            cur0 = sr[3]
        cur.append(i)
        cols += c
    if cur:
        chunks.append((cur0, cols, tuple(cur)))

    # schedule: chunks in order; lvl2 right after the last chunk feeding it;
    # MLP tiles as soon as their agg range is fully written.
    last_chunk_of_class = {}
    for ci_ch, (_, _, idxs) in enumerate(chunks):
        for i in idxs:
            last_chunk_of_class[subruns[i][6]] = ci_ch
    lvl2_after = {}
    for li, (nv, n, p0, vq0) in enumerate(lvl2):
        # class id of this lvl2 = the class whose p0 matches
        for ci, (d, cp0, cp1) in enumerate(classes):
            if cp0 == p0 and d > KMAX:
                lvl2_after.setdefault(last_chunk_of_class[ci], []).append(li)
                break

    # coverage: position p ready after event; direct positions after their
    # chunk, virtual positions after their lvl2
    ready_at = np.full(NPC, -1, np.int64)    # chunk index after which ready
    for ci_ch, (_, _, idxs) in enumerate(chunks):
        for i in idxs:
            d, nn, sp, _, _, kind, ci, jj, _ = subruns[i]
            if kind == 0:
                ready_at[sp:sp + nn] = np.maximum(ready_at[sp:sp + nn], ci_ch)
    for li, (nv, n, p0, vq0) in enumerate(lvl2):
        # ready after the chunk that its lvl2 is scheduled after
        for ci_ch, lis in lvl2_after.items():
            if li in lis:
                ready_at[p0:p0 + n] = ci_ch
    ready_at[zlo:zhi] = -1                   # ready from start

    # MLP emitted in groups of 4 tiles (stats batched per group)
    ngroups = -(-NMT // 4)
    sched = []
    g = 0
    for ci_ch in range(len(chunks)):
        sched.append(("c", ci_ch))
        for li in lvl2_after.get(ci_ch, []):
            sched.append(("v", li))
        while g < ngroups and np.all(
                ready_at[:min(4 * (g + 1) * MTILE, NPC)] <= ci_ch):
            sched.append(("g", g))
            g += 1
    while g < ngroups:
        sched.append(("g", g))
        g += 1

    return dict(
        E_pad=E_pad, NV=NV, zrange=(zlo, zhi),
        classes=tuple(map(tuple, classes)),
        subruns=tuple(map(tuple, subruns)),
        chunks=tuple(chunks),
        lvl2=tuple(lvl2),
        sched=tuple(sched),
    )


def _plan_key(plan):
    return (plan["E_pad"], plan["NV"], plan["zrange"], plan["subruns"],
            plan["chunks"], plan["lvl2"], plan["sched"],
            plan.get("b2z", False))


# ---------------------------------------------------------------------------
def _build_program(plan, reps=1, unroll=1):
    import contextlib
    _skip_birverifier()
    E_pad, NV = plan["E_pad"], plan["NV"]
    zlo, zhi = plan["zrange"]
    subruns, chunks, lvl2 = plan["subruns"], plan["chunks"], plan["lvl2"]
    sched = plan["sched"]

    nc = bass.Bass("TRN2", target_bir_lowering=False, debug=False,
                   num_devices=N_CORES)
    d_eattr = nc.dram_tensor("eattr", [128, E_pad],
                             F8 if EATTR_FP8 else F16,
                             kind="ExternalInput").ap()
    d_xT = nc.dram_tensor("xT", [128, NPC], F16, kind="ExternalInput").ap()
    d_w16 = nc.dram_tensor("w16", [128, W16C], F16,
                           kind="ExternalInput").ap()
    d_w32 = nc.dram_tensor("w32", [128, W32C], F32,
                           kind="ExternalInput").ap()
    d_outT = nc.dram_tensor("outT", [128, NPC], F16,
                            kind="ExternalOutput").ap()

    with tile.TileContext(nc) as tc:
        with tc.tile_pool(name="const", bufs=1) as constp, \
             tc.tile_pool(name="chp", bufs=3) as chp, \
             tc.tile_pool(name="trp", bufs=8) as trp, \
             tc.tile_pool(name="hp", bufs=10) as hp, \
             tc.tile_pool(name="sqp", bufs=4) as sqp, \
             tc.tile_pool(name="up", bufs=10) as up, \
             tc.tile_pool(name="omp", bufs=3) as omp, \
             tc.tile_pool(name="osbp", bufs=3) as osbp, \
             tc.tile_pool(name="statp", bufs=4) as statp, \
             tc.tile_pool(name="ps_h", bufs=3, space="PSUM") as ps_h, \
             tc.tile_pool(name="ps_st", bufs=3, space="PSUM") as ps_st, \
             tc.tile_pool(name="ps_po", bufs=2, space="PSUM") as ps_po:

            w16 = constp.tile([128, W16C], F16)
            nc.sync.dma_start(w16[:], d_w16)
            w32 = constp.tile([128, W32C], F32)
            nc.sync.dma_start(w32[:], d_w32)
            agg = constp.tile([128, NPC], F16)
            vscr = constp.tile([128, max(NV, 1)], F16)
            xfull = constp.tile([128, NPC], F16)

            if zhi > zlo:
                nc.gpsimd.memset(agg[:, zlo:zhi], 0.0)

            def W16(name, n=128):
                return w16[:, _O16[name]:_O16[name] + n]

            def W32(name):
                return w32[:, _O32[name]:_O32[name] + 1]


            uid = [0]

            def emit_tree(eng, blocks, dst, n):
                cur = list(blocks)
                if len(cur) == 1:
                    eng.tensor_copy(dst, cur[0])
                    return
                while len(cur) > 1:
                    nxt = []
                    i = 0
                    while i + 1 < len(cur):
                        if len(cur) == 2:
                            o = dst
                        else:
                            uid[0] += 1
                            t = trp.tile([128, n], F16,
                                         name=f"tr{uid[0]}", tag="tr")
                            o = t[:]
                        eng.tensor_tensor(o, cur[i], cur[i + 1], alu.add)
                        nxt.append(o)
                        i += 2
                    if i < len(cur):
                        nxt.append(cur[i])
                    cur = nxt

            def emit_chunk(ci_ch):
                src0, ncols, idxs = chunks[ci_ch]
                uid[0] += 1
                ch = chp.tile([128, ncols], F16, name=f"ch{uid[0]}", tag="ch")
                if EATTR_FP8:
                    # gpsimd DMA casts fp8 -> f16 in flight (HBM reads halved)
                    nc.gpsimd.dma_start(ch[:], d_eattr[:, src0:src0 + ncols])
                else:
                    nc.sync.dma_start(ch[:], d_eattr[:, src0:src0 + ncols])
                for i in idxs:
                    d, n, sp, src, dst, kind, cidx, jj, eng_id = subruns[i]
                    eng = nc.gpsimd if eng_id else nc.vector
                    off = src - src0
                    blocks = [ch[:, off + j * n: off + (j + 1) * n]
                              for j in range(d)]
                    if kind == 0:
                        out = agg[:, sp:sp + n]
                    else:
                        out = vscr[:, dst:dst + n]
                    emit_tree(eng, blocks, out, n)

            def emit_lvl2(li):
                nv, n, p0, vq0 = lvl2[li]
                blocks = [vscr[:, vq0 + j * n: vq0 + (j + 1) * n]
                          for j in range(nv)]
                emit_tree(nc.vector, blocks, agg[:, p0:p0 + n], n)

            U16 = mybir.dt.uint16

            def emit_mlp_group(g):
                m0 = 4 * g
                m1 = min(m0 + 4, NMT)
                G = m1 - m0
                pb = ps_st.tile([4, MTILE], F32, tag="st", name=f"pb_{g}")
                hs, us = [], []
                # phase 1: W1 + h/sq/u + variance rows for the whole group
                for r in range(G):
                    m = m0 + r
                    sl = slice(m * MTILE, (m + 1) * MTILE)
                    xt = xfull[:, sl]
                    aggm = agg[:, sl]
                    ph0 = ps_h.tile([128, MTILE], F32, tag="ph",
                                    name=f"ph0_{m}")
                    ph1 = ps_h.tile([128, MTILE], F32, tag="ph",
                                    name=f"ph1_{m}")
                    nc.tensor.matmul(ph0[:], W16("w1a0"), xt, start=True,
                                     stop=False)
                    nc.tensor.matmul(ph0[:], W16("w1b0"), aggm, start=False,
                                     stop=True)
                    nc.tensor.matmul(ph1[:], W16("w1a1"), xt, start=True,
                                     stop=False)
                    nc.tensor.matmul(ph1[:], W16("w1b1"), aggm, start=False,
                                     stop=True)
                    h0 = hp.tile([128, MTILE], F16, tag="h", name=f"h0_{m}")
                    h1 = hp.tile([128, MTILE], F16, tag="h", name=f"h1_{m}")
                    nc.scalar.activation(h0[:], ph0[:], act.Identity,
                                         bias=W32("b1c0"))
                    nc.scalar.activation(h1[:], ph1[:], act.Identity,
                                         bias=W32("b1c1"))
                    sq0 = sqp.tile([128, MTILE], F16, tag="sq",
                                   name=f"sq0_{m}")
                    sq1 = sqp.tile([128, MTILE], F16, tag="sq",
                                   name=f"sq1_{m}")
                    nc.vector.tensor_tensor(sq0[:], h0[:], h0[:], alu.mult)
                    nc.vector.tensor_tensor(sq1[:], h1[:], h1[:], alu.mult)
                    u0 = up.tile([128, MTILE], F16, tag="u", name=f"u0_{m}")
                    u1 = up.tile([128, MTILE], F16, tag="u", name=f"u1_{m}")
                    # |h| on f16 = clear sign bit (no ABS in mybir's alu set;
                    # bitvec ops need integer dtypes -> bitcast to u16)
                    nc.vector.tensor_scalar(u0[:].bitcast(U16),
                                            h0[:].bitcast(U16), 0x7FFF, None,
                                            alu.bitwise_and)
                    nc.vector.tensor_scalar(u1[:].bitcast(U16),
                                            h1[:].bitcast(U16), 0x7FFF, None,
                                            alu.bitwise_and)
                    er = w16[:, _O16["er"] + 4 * r:_O16["er"] + 4 * r + 4]
                    nc.tensor.matmul(pb[0:4, :], er, sq0[:],
                                     start=(r == 0), stop=False)
                    nc.tensor.matmul(pb[0:4, :], er, sq1[:],
                                     start=False, stop=(r == G - 1))
                    hs.append((h0, h1))
                    us.append((u0, u1))
                # batched stats: rstd rows for the group
                yrow = statp.tile([4, MTILE], F32, tag="yr", name=f"yr_{g}")
                nc.scalar.activation(yrow[0:G, :], pb[0:G, :], act.Ln,
                                     scale=1.0 / 256.0,
                                     bias=w32[0:G, _O32["epsc"]:
                                              _O32["epsc"] + 1])
                rrow = statp.tile([4, MTILE], F16, tag="rr", name=f"rr_{g}")
                nc.scalar.activation(rrow[0:G, :], yrow[0:G, :], act.Exp,
                                     scale=-0.5)
                # phase 2: broadcast rstd, W2, scale, bias, store
                for r in range(G):
                    m = m0 + r
                    sl = slice(m * MTILE, (m + 1) * MTILE)
                    h0, h1 = hs[r]
                    u0, u1 = us[r]
                    sel = w16[0:4, _O16["sel"] + 128 * r:
                              _O16["sel"] + 128 * (r + 1)]
                    pr = ps_st.tile([128, MTILE], F32, tag="st",
                                    name=f"pr_{m}")
                    nc.tensor.matmul(pr[:], sel, rrow[0:4, :], start=True,
                                     stop=True)
                    rr_sb = omp.tile([128, MTILE], F16, tag="rrs",
                                     name=f"rrs_{m}")
                    nc.scalar.activation(rr_sb[:], pr[:], act.Copy)
                    po = ps_po.tile([128, MTILE], F32, tag="po",
                                    name=f"po_{m}")
                    nc.tensor.matmul(po[:], W16("w2v0"), h0[:], start=True,
                                     stop=False)
                    nc.tensor.matmul(po[:], W16("w2v1"), h1[:], start=False,
                                     stop=False)
                    nc.tensor.matmul(po[:], W16("w2u0"), u0[:], start=False,
                                     stop=False)
                    nc.tensor.matmul(po[:], W16("w2u1"), u1[:], start=False,
                                     stop=True)
                    osb = osbp.tile([128, MTILE], F16, tag="osb",
                                    name=f"osb_{m}")
                    if plan.get("b2z"):
                        # b2 == 0: the rstd multiply writes the output tile
                        nc.vector.tensor_tensor(osb[:], po[:], rr_sb[:],
                                                alu.mult)
                    else:
                        om = omp.tile([128, MTILE], F16, tag="om",
                                      name=f"om_{m}")
                        nc.vector.tensor_tensor(om[:], po[:], rr_sb[:],
                                                alu.mult)
                        nc.vector.tensor_scalar(osb[:], om[:], W32("b2c"),
                                                None, alu.add)
                    nc.sync.dma_start(d_outT[:, sl], osb[:])

            def emit_rep():
                # interleave the x load behind the first edge chunks so the
                # edge stream (the critical path) starts immediately
                xi = [0]

                def emit_x():
                    if xi[0] < 8:
                        xw = NPC // 8
                        sl = slice(xi[0] * xw, (xi[0] + 1) * xw)
                        nc.sync.dma_start(xfull[:, sl], d_xT[:, sl])
                        xi[0] += 1

                for kind_it, idx in sched:
                    if kind_it == "c":
                        emit_chunk(idx)
                        emit_x()
                    elif kind_it == "v":
                        emit_lvl2(idx)
                    else:
                        emit_mlp_group(idx)
                while xi[0] < 8:
                    emit_x()

            with nc.allow_low_precision("f16 pipeline"):
                if unroll > 1:
                    for _ in range(unroll):
                        emit_rep()
                else:
                    rep_ctx = (tc.For_i(0, reps, 1) if reps > 1
                               else contextlib.nullcontext())
                    rep_ctx.__enter__()
                    emit_rep()
                    rep_ctx.__exit__(None, None, None)

    _split_multi_waits(nc)
    _fuse_single_waits(nc)
    return nc


# ---------------------------------------------------------------------------
class _Runner:
    """Persistent executor: jit once, keep inputs on device."""

    def __init__(self, nc):
        import jax
        from jax.experimental.shard_map import shard_map
        from jax.sharding import Mesh, PartitionSpec, NamedSharding
        from concourse import bass2jax
        from concourse import mybir as _mb

        bass2jax.install_neuronx_cc_hook()
        self.nc = nc
        in_names, out_names, out_avals = [], [], []
        partition_name = (nc.partition_id_tensor.name
                          if nc.partition_id_tensor else None)
        for alloc in nc.m.functions[0].allocations:
            if not isinstance(alloc, _mb.MemoryLocationSet):
                continue
            name = alloc.memorylocations[0].name
            if alloc.kind == "ExternalInput":
                if name != partition_name:
                    in_names.append(name)
            elif alloc.kind == "ExternalOutput":
                out_names.append(name)
                out_avals.append(jax.core.ShapedArray(
                    tuple(alloc.tensor_shape), _mb.dt.np(alloc.dtype)))
        self.in_names, self.out_names, self.out_avals = \
            in_names, out_names, out_avals
        n_params, n_outs = len(in_names), len(out_avals)
        all_in = list(in_names) + list(out_names)
        if partition_name is not None:
            all_in.append(partition_name)

        def _body(*args):
            operands = list(args)
            if partition_name is not None:
                operands.append(bass2jax.partition_id_tensor())
            return tuple(bass2jax._bass_exec_p.bind(
                *operands,
                out_avals=tuple(out_avals),
                in_names=tuple(all_in),
                out_names=tuple(out_names),
                lowering_input_output_aliases=(),
                sim_require_finite=True,
                sim_require_nnan=True,
                nc=nc,
            ))

        devices = jax.devices()[:N_CORES]
        mesh = Mesh(np.asarray(devices), ("core",))
        self.mesh = mesh
        self.sharding = NamedSharding(mesh, PartitionSpec("core"))
        in_specs = (PartitionSpec("core"),) * (n_params + n_outs)
        out_specs = (PartitionSpec("core"),) * n_outs
        donate = tuple(range(n_params, n_params + n_outs))
        self.fn = jax.jit(
            shard_map(_body, mesh=mesh, in_specs=in_specs,
                      out_specs=out_specs, check_rep=False),
            donate_argnums=donate, keep_unused=True)
        self._zero = jax.jit(
            lambda: tuple(
                jax.numpy.zeros((N_CORES * a.shape[0], *a.shape[1:]), a.dtype)
                for a in out_avals),
            out_shardings=tuple(self.sharding for _ in out_avals))
        self._dev_inputs = None
        self._dev_key = None

    def put_inputs(self, in_maps):
        import jax
        key = tuple(id(m[n]) for m in in_maps for n in self.in_names)
        if self._dev_key == key and self._dev_inputs is not None:
            return
        concat = [np.concatenate([np.asarray(m[n]) for m in in_maps], axis=0)
                  for n in self.in_names]
        self._dev_inputs = [jax.device_put(a, self.sharding) for a in concat]
        for a in self._dev_inputs:
            a.block_until_ready()
        self._dev_key = key

    def execute(self):
        zeros = self._zero()
        outs = self.fn(*self._dev_inputs, *zeros)
        return outs

    def run(self, in_maps):
        self.put_inputs(in_maps)
        outs = self.execute()
        res = []
        for c in range(N_CORES):
            res.append({
                name: np.asarray(outs[i]).reshape(
                    N_CORES, *self.out_avals[i].shape)[c]
                for i, name in enumerate(self.out_names)})
        return res

    def time_once(self):
        import time as _t
        zeros = self._zero()
        for z in zeros:
            z.block_until_ready()
        t0 = _t.perf_counter()
        outs = self.fn(*self._dev_inputs, *zeros)
        for o in outs:
            o.block_until_ready()
        return _t.perf_counter() - t0


_CACHE = {}


def _prepare(x, edge_index, edge_attr, W1, b1, gamma, beta, prelu_a, W2, b2):
    N, E = x.shape[0], edge_attr.shape[0]
    assert np.all(np.asarray(beta) == 0.0), "kernel specialized for beta=0"
    x = np.asarray(x, np.float32)
    edge_attr = np.asarray(edge_attr, np.float32)
    col = np.asarray(edge_index)[1].astype(np.int64)

    cnt = np.bincount(col, minlength=N_PAD).astype(np.int64)
    inv = (1.0 / np.maximum(cnt, 1.0)).astype(np.float32)

    # Custom class order tuned for pipeline ramp: zero-degree first (agg is
    # memset -> MLP group 0 starts immediately), mid degrees next, the
    # two-level virtual classes (d>KMAX, longest latency chains) in the
    # middle, and the shallowest trees (d=3,2,1) last so the final MLP
    # group's agg is ready right after the last (small) chunk.
    dmax = int(cnt.max())
    rank = np.empty(dmax + 1, np.int64)
    for d in range(dmax + 1):
        if d == 0:
            rank[d] = 0
        elif 4 <= d <= KMAX:
            rank[d] = 1 + (KMAX - d)           # 8,7,6,5,4
        elif d > KMAX:
            rank[d] = 100 + d                  # virtuals, middle
        else:
            rank[d] = 1000 + (3 - d)           # 3,2,1 at the tail
    order = np.argsort(rank[cnt], kind="stable")
    r = np.arange(N_PAD)
    new_of_old = np.empty(N_PAD, np.int64)
    new_of_old[order] = (r % N_CORES) * NPC + r // N_CORES
    D = cnt[order].reshape(NPC, N_CORES).max(axis=1)   # shared profile

    plan = _make_plan(D)
    # NB: skipping the (zero) b2-add measurably HURTS the schedule - the ts
    # decouples the psum-reading multiply from the out-DMA. Keep it.
    plan["b2z"] = False
    E_pad = plan["E_pad"]

    # per-edge (core, pos, slot)
    new = new_of_old[col]
    core_e = new // NPC
    pos_e = new % NPC
    order_e = np.argsort(new, kind="stable")
    sn = new[order_e]
    change = np.r_[True, sn[1:] != sn[:-1]]
    startidx = np.maximum.accumulate(np.where(change, np.arange(E), 0))
    slot = np.empty(E, np.int64)
    slot[order_e] = np.arange(E) - startidx

    # per-edge column via sub-run lookup
    classes = plan["classes"]
    colx = np.full(E, -1, np.int64)
    # precompute virtual split (j, e_in) for edges in split classes
    jj_e = np.zeros(E, np.int64)
    ein_e = slot.copy()
    for ci, (d, p0, p1) in enumerate(classes):
        if d <= KMAX:
            continue
        nv = -(-d // KMAX)
        base = d // nv
        rem = d % nv
        cmask = (pos_e >= p0) & (pos_e < p1)
        e = slot[cmask]
        b1_ = rem * (base + 1)
        jv = np.where(e < b1_, e // (base + 1), rem + (e - b1_) // base)
        ev = np.where(e < b1_, e % (base + 1), (e - b1_) % base)
        jj_e[cmask] = jv
        ein_e[cmask] = ev
    for d, n, sp, src, dst, kind, ci, jj, eng in plan["subruns"]:
        m = (pos_e >= sp) & (pos_e < sp + n) & (jj_e == jj)
        colx[m] = src + ein_e[m] * n + (pos_e[m] - sp)
    assert (colx >= 0).all()

    edtype = mybir.dt.np(F8) if EATTR_FP8 else np.float16
    val = (edge_attr * inv[col][:, None]).astype(edtype)
    buf = np.zeros((N_CORES, E_pad, H), edtype)
    buf[core_e, colx] = val
    eattrT = np.ascontiguousarray(buf.transpose(0, 2, 1))

    xp = np.zeros((N_PAD, H), np.float32)
    xp[new_of_old[:N]] = x
    xT = np.ascontiguousarray(
        xp.reshape(N_CORES, NPC, H).transpose(0, 2, 1)).astype(np.float16)

    w16, w32 = _build_wbufs(W1, b1, gamma, beta, prelu_a, W2, b2)

    in_maps = [
        {"eattr": eattrT[c], "xT": xT[c], "w16": w16, "w32": w32}
        for c in range(N_CORES)
    ]
    return plan, in_maps, new_of_old


def get_runner(plan, reps=1):
    ck = (_plan_key(plan), reps)
    runner = _CACHE.get(ck)
    if runner is None:
        nc = _build_program(plan, reps=reps)
        runner = _Runner(nc)
        _CACHE[ck] = runner
    return runner


def kernel(x, edge_index, edge_attr, W1, b1, gamma, beta, prelu_a, W2, b2,
           **_unused):
    N = x.shape[0]
    plan, in_maps, new_of_old = _prepare(x, edge_index, edge_attr, W1, b1,
                                         gamma, beta, prelu_a, W2, b2)
    runner = get_runner(plan)
    res = runner.run(in_maps)
    outT = np.stack([r["outT"] for r in res])           # [8,128,NPC] f16
    out = outT.transpose(0, 2, 1).reshape(N_PAD, H).astype(np.float32)
    out = out[new_of_old[:N]]
    return np.ascontiguousarray(out)


if __name__ == "__main__":
    rng = np.random.default_rng(0)
    N, E = N_PAD, 60000
    x = rng.standard_normal((N, H), dtype=np.float32)
    ei = rng.integers(0, N, size=(2, E)).astype(np.int64)
    ea = rng.standard_normal((E, H), dtype=np.float32)
    W1 = rng.standard_normal((2 * H, 2 * H), dtype=np.float32) / 16
    b1 = np.zeros(2 * H, np.float32)
    g = np.ones(2 * H, np.float32)
    be = np.zeros(2 * H, np.float32)
    a = np.full(1, 0.25, np.float32)
    W2 = rng.standard_normal((2 * H, H), dtype=np.float32) / 16
    b2 = np.zeros(H, np.float32)
    out = kernel(x, ei, ea, W1, b1, g, be, a, W2, b2)
    print("out", out.shape, out.dtype, np.abs(out).mean())

